# revision 3
# baseline (speedup 1.0000x reference)
"""Trainium2 Bass kernel for nn_Clustered_Attention_Chunking — v2.

Same math as the baseline (batched per-chunk self-attention + residual
layernorm; host-side sort/unsort only when the mask is nonzero), rebuilt
around the measured bottlenecks of the 1.20ms baseline:

  * PE (TensorMatrix) was busy 950us of 1201us but ~40% of matmul time ran
    at the 1.2GHz p-state (idle gaps reset the DVFS ramp) — the new
    schedule interleaves next-macro projection matmuls into the attention
    stream so the PE never waits on the softmax chain.
  * x was loaded twice + transposed via 16 SBUF-SBUF DMAs per macro
    (289us of SP/sequencer time): xT is now precomputed on the host and
    DMA'd directly (bf16), natural x loads once (f32, residual).
  * Weights are host-cast to bf16: no staging copies, halved startup DMA.
  * ACT did 456us of psum->sbuf copies + 82us of Exp<->Sqrt table churn:
    softmax sums/normalize moved to the (idle) GPSIMD engine, ctx/pts
    copies to DVE, and the layernorm Sqrt is batched once per 4 macros.

Per-core layout (data parallel, 2048 seqs / 8 cores, no collectives):
macro = 512 tokens (8 seqs); per macro the PE runs 45056 matmul rows
(proj q/k/v/out 4x8192 + attention 12288) ~= 18.8us at 2.4GHz; ACT ~10us,
DVE ~13us, GPSIMD ~9us all hide behind it.
"""

import numpy as np

H = 8
E = 512
C = 64
N_FULL = 2048
N_CORES = 8
NSH = N_FULL // N_CORES       # 256 sequences per core
T_FULL = NSH * C              # 16384 tokens per core
TM = 512                      # tokens per macro-block
EPS = 1e-12

_CACHE = {}


def _build_program(use_mask, use_bq, use_bk, use_bv, use_bd, T=T_FULL):
    from collections import deque
    from contextlib import ExitStack

    import ml_dtypes
    import concourse.bass as bass
    import concourse.mybir as mybir
    import concourse.tile as tile
    from concourse import bacc

    f32 = mybir.dt.float32
    bf16 = mybir.dt.bfloat16
    AF = mybir.ActivationFunctionType
    ALU = mybir.AluOpType

    N_MACRO = T // TM
    G = min(4, N_MACRO)        # layernorm sqrt batch (macros per quad)
    assert N_MACRO % G == 0

    nc = bacc.Bacc("TRN2")

    x_d = nc.dram_tensor("x", [T, E], f32, kind="ExternalInput")
    xt_d = nc.dram_tensor("xt", [E, T], bf16, kind="ExternalInput")
    wq_d = nc.dram_tensor("wqt", [E, E], bf16, kind="ExternalInput")
    wk_d = nc.dram_tensor("wkt", [E, E], bf16, kind="ExternalInput")
    wv_d = nc.dram_tensor("wvt", [E, E], bf16, kind="ExternalInput")
    wd_d = nc.dram_tensor("wdt", [E, E], bf16, kind="ExternalInput")
    out_d = nc.dram_tensor("out", [T, E], f32, kind="ExternalOutput")
    mask_d = bq_d = bk_d = bv_d = bd_d = None
    if use_mask:
        mask_d = nc.dram_tensor("mask", [T, C], f32, kind="ExternalInput")
    if use_bq:
        bq_d = nc.dram_tensor("bq", [E], f32, kind="ExternalInput")
    if use_bk:
        bk_d = nc.dram_tensor("bk", [E], f32, kind="ExternalInput")
    if use_bv:
        bv_d = nc.dram_tensor("bv", [E], f32, kind="ExternalInput")
    if use_bd:
        bd_d = nc.dram_tensor("bdb", [128, E], f32, kind="ExternalInput")

    id64_np = np.tile(np.eye(64, dtype=np.float32), (2, 1)).astype(ml_dtypes.bfloat16)
    id64_d = nc.inline_tensor(id64_np, name="id64")

    def bcast_last(ap2d, n):
        """[128, k] AP -> [128, k, n] with stride-0 innermost dim."""
        return bass.AP(ap2d.tensor, ap2d.offset, list(ap2d.ap) + [[0, n]])

    with tile.TileContext(nc) as tc, ExitStack() as ctx:
        consts = ctx.enter_context(tc.tile_pool(name="consts", bufs=1))

        # Weights, host-side pre-transposed + bf16: w*T[e, e'] = W[e', e],
        # tiled [p, a, e'] with row index e = a*128 + p.  Direct DMA.
        w_sb = {}
        for nm, dd in (("q", wq_d), ("k", wk_d), ("v", wv_d), ("d", wd_d)):
            t = consts.tile([128, 4, E], bf16, tag=f"w{nm}", name=f"w{nm}")
            nc.sync.dma_start(t[:], dd[:].rearrange("(a p) e -> p a e", p=128))
            w_sb[nm] = t

        id64 = consts.tile([128, 64], bf16, tag="id64", name="id64")
        nc.sync.dma_start(id64[:], id64_d[:])
        eps_t = consts.tile([128, 1], f32, tag="eps", name="eps")
        nc.vector.memset(eps_t[:], EPS)

        bias_sb = {}
        for nm, dd in (("q", bq_d), ("k", bk_d), ("v", bv_d)):
            if dd is not None:
                t = consts.tile([128, 4], f32, tag=f"b{nm}", name=f"b{nm}")
                nc.sync.dma_start(t[:], dd[:].rearrange("(a p) -> p a", p=128))
                bias_sb[nm] = t
        if bd_d is not None:
            t = consts.tile([128, E], f32, tag="bd", name="bd")
            nc.sync.dma_start(t[:], bd_d[:])
            bias_sb["d"] = t

        # SBUF pools
        p_xt = ctx.enter_context(tc.tile_pool(name="p_xt", bufs=4))
        p_x = ctx.enter_context(tc.tile_pool(name="p_x", bufs=4))
        p_qk = ctx.enter_context(tc.tile_pool(name="p_qk", bufs=4))
        p_v = ctx.enter_context(tc.tile_pool(name="p_v", bufs=2))
        p_ct = ctx.enter_context(tc.tile_pool(name="p_ct", bufs=2))
        p_pr = ctx.enter_context(tc.tile_pool(name="p_pr", bufs=12))
        p_sm = ctx.enter_context(tc.tile_pool(name="p_sm", bufs=10))
        p_mv = ctx.enter_context(tc.tile_pool(name="p_mv", bufs=2))
        p_h = ctx.enter_context(tc.tile_pool(name="p_h", bufs=G + 2))
        p_msk = (
            ctx.enter_context(tc.tile_pool(name="p_msk", bufs=3)) if use_mask else None
        )

        # PSUM: pp = [128,512] f32 (1 bank) x3 shared by proj + out-proj;
        # pa = [128,4,64] f32 (1 bank) x4 for scores/ctx; pb = transposes.
        pp = ctx.enter_context(tc.tile_pool(name="pp", bufs=3, space="PSUM"))
        pa = ctx.enter_context(tc.tile_pool(name="pa", bufs=4, space="PSUM"))
        pb = ctx.enter_context(tc.tile_pool(name="pb", bufs=1, space="PSUM"))

        tiles_in = {}

        def dma_in(m):
            """Issue input DMAs for macro m: xT (bf16) and natural x (f32)."""
            t0 = m * TM
            xt = p_xt.tile([128, 4, TM], bf16, tag="xt", name="xt")
            nc.sync.dma_start(
                xt[:], xt_d[:, t0 : t0 + TM].rearrange("(a p) t -> p a t", p=128)
            )
            xn = p_x.tile([128, 4, E], f32, tag="xn", name="xn")
            nc.sync.dma_start(
                xn[:], x_d[t0 : t0 + TM, :].rearrange("(a p) e -> p a e", p=128)
            )
            msk = None
            if use_mask:
                msk = p_msk.tile([128, 4, C], f32, tag="msk", name="msk")
                nc.sync.dma_start(
                    msk[:], mask_d[t0 : t0 + TM, :].rearrange("(a p) c -> p a c", p=128)
                )
            tiles_in[m] = (xt, xn, msk)

        qkv = {}

        def make_proj_chunks(m):
            """Build 12 emission thunks for macro m's q/k/v projections.
            Each chunk: 4 accumulating matmuls into one PSUM bank + one
            ACT psum->sbuf bf16 copy."""
            xt = tiles_in[m][0]
            q_t = p_qk.tile([128, 4, TM], bf16, tag="qT", name="qT")
            k_t = p_qk.tile([128, 4, TM], bf16, tag="kT", name="kT")
            v_t = p_v.tile([128, 4, E], bf16, tag="v", name="v")
            qkv[m] = (q_t, k_t, v_t)
            chunks = []

            def qk_chunk(nm, dst, c):
                def emit():
                    ps = pp.tile([128, TM], f32, tag="proj", name="proj")
                    for ec in range(4):
                        nc.tensor.matmul(
                            ps[:],
                            w_sb[nm][:, ec, c * 128 : (c + 1) * 128],
                            xt[:, ec, :],
                            start=(ec == 0),
                            stop=(ec == 3),
                        )
                    if nm in bias_sb:
                        nc.scalar.activation(
                            dst[:, c, :], ps[:], AF.Identity,
                            bias=bias_sb[nm][:, c : c + 1],
                        )
                    else:
                        nc.scalar.copy(dst[:, c, :], ps[:])
                return emit

            def v_chunk(t4):
                def emit():
                    ps = pp.tile([128, E], f32, tag="proj", name="proj")
                    for ec in range(4):
                        nc.tensor.matmul(
                            ps[:],
                            xt[:, ec, t4 * 128 : (t4 + 1) * 128],
                            w_sb["v"][:, ec, :],
                            start=(ec == 0),
                            stop=(ec == 3),
                        )
                    nc.scalar.copy(v_t[:, t4, :], ps[:])
                return emit

            for c in range(4):
                chunks.append(qk_chunk("q", q_t, c))
                chunks.append(qk_chunk("k", k_t, c))
            for t4 in range(4):
                chunks.append(v_chunk(t4))
            return chunks

        def scores_softmax(m, p4):
            """scores (PE, quad-packed) -> exp (ACT) -> sums (GPSIMD) ->
            recip (DVE) -> normalized probs (GPSIMD)."""
            q_t, k_t, _ = qkv[m]
            msk = tiles_in[m][2]
            ps_s = [
                pa.tile([128, 4, 64], f32, tag="small", name="ps_s")
                for _ in (0, 1)
            ]
            # Quadrant order (hb, sb): diagonal-complementary pairs so
            # consecutive matmuls occupy disjoint PE row/col groups and
            # overlap like the transposes do (serial pattern costs ~143ns
            # per 64x64 matmul vs ~30ns overlapped).
            for c in range(4):
                for hb, sb_ in ((0, 0), (1, 1), (0, 1), (1, 0)):
                    hsl = slice(hb * 64, (hb + 1) * 64)
                    tsl = slice(p4 * 128 + sb_ * 64, p4 * 128 + (sb_ + 1) * 64)
                    nc.tensor.matmul(
                        ps_s[hb][sb_ * 64 : (sb_ + 1) * 64, c, :],
                        q_t[hsl, c, tsl],
                        k_t[hsl, c, tsl],
                        start=True,
                        stop=True,
                    )
            if use_mask:
                for hb in (0, 1):
                    for c in range(4):
                        nc.vector.tensor_add(
                            ps_s[hb][:, c, :], ps_s[hb][:, c, :], msk[:, p4, :]
                        )
            probs = [
                p_pr.tile([128, 4, 64], bf16, tag="probs", name="probs")
                for _ in (0, 1)
            ]
            sums = p_sm.tile([128, 2, 4], f32, tag="sums", name="sums")
            for hb in (0, 1):
                nc.scalar.activation(probs[hb][:], ps_s[hb][:], AF.Exp, scale=0.125)
                nc.vector.tensor_reduce(
                    sums[:, hb, :], probs[hb][:], axis=mybir.AxisListType.X,
                    op=ALU.add,
                )
            recip = p_sm.tile([128, 2, 4], f32, tag="recip", name="recip")
            nc.vector.reciprocal(recip[:], sums[:])
            pn = [
                p_pr.tile([128, 4, 64], bf16, tag="pn", name="pn")
                for _ in (0, 1)
            ]
            for hb in (0, 1):
                nc.gpsimd.tensor_tensor(
                    pn[hb][:], probs[hb][:], bcast_last(recip[:, hb, :], 64),
                    op=ALU.mult,
                )
            return pn

        def trans(pn):
            """64x64 PE transposes of normalized probs; psum -> sbuf on DVE."""
            ps_pt = pb.tile([128, 2, 4, 64], bf16, tag="pt", name="ps_pt")
            for hb in (0, 1):
                for c in range(4):
                    for sb_ in (0, 1):
                        ssl = slice(sb_ * 64, (sb_ + 1) * 64)
                        nc.tensor.transpose(
                            ps_pt[ssl, hb, c, :], pn[hb][ssl, c, :], id64[ssl, :]
                        )
            pts = p_pr.tile([128, 2, 4, 64], bf16, tag="pts", name="pts")
            nc.vector.tensor_copy(pts[:], ps_pt[:])
            return pts

        def ctx_out(m, p4, pts, ctxT):
            """ctx^T (PE) -> ctxT sbuf (DVE)."""
            _, _, v_t = qkv[m]
            ps_c = [
                pa.tile([128, 4, 64], f32, tag="small", name="ps_c")
                for _ in (0, 1)
            ]
            # Same diagonal-pair quadrant ordering as scores (row group =
            # sb token half, col group = hb head half).
            for c in range(4):
                for sb_, hb in ((0, 0), (1, 1), (0, 1), (1, 0)):
                    ssl = slice(sb_ * 64, (sb_ + 1) * 64)
                    hsl = slice(hb * 64, (hb + 1) * 64)
                    nc.tensor.matmul(
                        ps_c[sb_][hsl, c, :],
                        v_t[ssl, p4, (2 * c + hb) * 64 : (2 * c + hb + 1) * 64],
                        pts[ssl, hb, c, :],
                        start=True,
                        stop=True,
                    )
            for sb_ in (0, 1):
                dst = ctxT[:, :, p4 * 128 + sb_ * 64 : p4 * 128 + (sb_ + 1) * 64]
                if "v" in bias_sb:
                    for c in range(4):
                        nc.vector.tensor_scalar_add(
                            dst[:, c, :], ps_c[sb_][:, c, :],
                            bias_sb["v"][:, c : c + 1],
                        )
                else:
                    nc.vector.tensor_copy(dst, ps_c[sb_][:])

        h_tiles = {}
        mv_cur = [None]

        def outproj(m, ctxT):
            """out-proj (PE) -> +residual (DVE) -> bn stats (DVE)."""
            xn = tiles_in[m][1]
            h = p_h.tile([128, 4, E], f32, tag="h", name="h")
            h_tiles[m] = h
            if m % G == 0:
                mv_cur[0] = p_mv.tile([128, 2, 4 * G], f32, tag="mv", name="mv")
            mv = mv_cur[0]
            for t4 in range(4):
                ps_o = pp.tile([128, E], f32, tag="proj", name="proj")
                for c in range(4):
                    nc.tensor.matmul(
                        ps_o[:],
                        ctxT[:, c, t4 * 128 : (t4 + 1) * 128],
                        w_sb["d"][:, c, :],
                        start=(c == 0),
                        stop=(c == 3),
                    )
                nc.vector.tensor_add(h[:, t4, :], ps_o[:], xn[:, t4, :])
                if "d" in bias_sb:
                    nc.vector.tensor_add(h[:, t4, :], h[:, t4, :], bias_sb["d"][:])
                stats = p_sm.tile([128, 6], f32, tag="stats", name="stats")
                nc.vector.bn_stats(stats[:], h[:, t4, :])
                idx = (m % G) * 4 + t4
                nc.vector.bn_aggr(mv[:, :, idx : idx + 1], stats[:])

        def ln_quad(m):
            """Batched layernorm tail for macros m-G+1..m: one ACT Sqrt
            (single Exp<->Sqrt table swap per quad), then in-place affine
            + output DMA per macro."""
            mv = mv_cur[0]
            std = p_sm.tile([128, 4 * G], f32, tag="std", name="std")
            nc.scalar.activation(
                std[:], mv[:, 1, :], AF.Sqrt, bias=eps_t[:, 0:1], scale=1.0
            )
            rstd = p_sm.tile([128, 4 * G], f32, tag="rstd", name="rstd")
            nc.vector.reciprocal(rstd[:], std[:])
            negmr = p_sm.tile([128, 4 * G], f32, tag="negmr", name="negmr")
            nc.vector.tensor_mul(negmr[:], mv[:, 0, :], rstd[:])
            for mq in range(m - G + 1, m + 1):
                h = h_tiles.pop(mq)
                for t4 in range(4):
                    idx = (mq % G) * 4 + t4
                    nc.vector.tensor_scalar(
                        h[:, t4, :], h[:, t4, :],
                        rstd[:, idx : idx + 1], negmr[:, idx : idx + 1],
                        ALU.mult, ALU.subtract,
                    )
                t0 = mq * TM
                nc.sync.dma_start(
                    out_d[t0 : t0 + TM, :].rearrange("(a p) e -> p a e", p=128),
                    h[:],
                )

        # ---- main schedule ----
        dma_in(0)
        if N_MACRO > 1:
            dma_in(1)
        for chk in make_proj_chunks(0):
            chk()

        for m in range(N_MACRO):
            if m + 2 < N_MACRO:
                dma_in(m + 2)
            pending = deque(make_proj_chunks(m + 1)) if m + 1 < N_MACRO else deque()

            def bf(n):
                for _ in range(n):
                    if pending:
                        pending.popleft()()

            ctxT = p_ct.tile([128, 4, TM], bf16, tag="ctxT", name="ctxT")
            # attention, 3-stage software pipeline with projection backfill:
            # PE runs scores(p4) / trans(p4-1) / ctx(p4-2) with next-macro
            # projection chunks spliced between stages so it never idles
            # while the ACT/GPSIMD/DVE softmax chain catches up.
            pn_l = [None] * 4
            pts_l = [None] * 4
            for p4 in range(4):
                pn_l[p4] = scores_softmax(m, p4)
                bf(2)
                if p4 >= 1:
                    pts_l[p4 - 1] = trans(pn_l[p4 - 1])
                    bf(1)
                if p4 >= 2:
                    ctx_out(m, p4 - 2, pts_l[p4 - 2], ctxT)
                    bf(1)
            pts_l[3] = trans(pn_l[3])
            bf(1)
            ctx_out(m, 2, pts_l[2], ctxT)
            bf(1)
            ctx_out(m, 3, pts_l[3], ctxT)
            while pending:
                pending.popleft()()
            outproj(m, ctxT)
            del tiles_in[m]
            del qkv[m]
            if m % G == G - 1:
                ln_quad(m)

    nc.compile()
    return nc


def _ensure_ntff_hook():
    """bass_utils' trace path does `from antenv.axon_hooks import ...`,
    which this container's antenv lacks.  Provide it, wired to the axon
    PJRT .so via ctypes (mirrors trn_agent_boot._ntff_profile_via_ctypes),
    so trace=True works; degrade to a None hook otherwise."""
    import sys
    import types

    try:
        import antenv.axon_hooks  # noqa: F401

        return
    except ImportError:
        pass
    mod = types.ModuleType("antenv.axon_hooks")
    state = {"hook": None}
    mod.set_axon_ntff_profile_hook = lambda h: state.__setitem__("hook", h)
    mod.get_axon_ntff_profile_hook = lambda: state["hook"]
    try:
        import antenv

        antenv.axon_hooks = mod
    except ImportError:
        pass
    sys.modules["antenv.axon_hooks"] = mod

    so_path = "/opt/axon/libaxon_pjrt.so"
    try:
        import importlib.util
        import os

        boot_py = None
        for base in (os.environ.get("AXON_SITE_DIR", "/root/.axon_site"),):
            cand = os.path.join(base, "trn_agent_boot", "trn_boot.py")
            if os.path.exists(cand):
                boot_py = cand
        if boot_py and os.path.exists(so_path):
            spec = importlib.util.spec_from_file_location("_trn_boot_hook", boot_py)
            tb = importlib.util.module_from_spec(spec)
            spec.loader.exec_module(tb)
            state["hook"] = tb._ntff_profile_via_ctypes(so_path)
    except Exception:
        state["hook"] = None


def kernel(
    seq,
    attention_mask,
    cluster_id,
    Wq,
    bq,
    Wk,
    bk,
    Wv,
    bv,
    Wd,
    bd,
    ln_w,
    ln_b,
):
    _ensure_ntff_hook()
    import ml_dtypes
    import concourse.bass_utils as bass_utils

    seq = np.ascontiguousarray(np.asarray(seq, dtype=np.float32))
    attention_mask = np.asarray(attention_mask, dtype=np.float32)
    use_mask = bool(np.any(attention_mask))
    Wq = np.asarray(Wq, np.float32)
    Wk = np.asarray(Wk, np.float32)
    Wv = np.asarray(Wv, np.float32)
    Wd = np.asarray(Wd, np.float32)
    bq = np.asarray(bq, np.float32)
    bk = np.asarray(bk, np.float32)
    bv = np.asarray(bv, np.float32)
    bd = np.asarray(bd, np.float32)
    ln_w = np.asarray(ln_w, np.float32)
    ln_b = np.asarray(ln_b, np.float32)
    use_bq, use_bk = bool(np.any(bq)), bool(np.any(bk))
    use_bv, use_bd = bool(np.any(bv)), bool(np.any(bd))

    key = (use_mask, use_bq, use_bk, use_bv, use_bd)
    if key not in _CACHE:
        _CACHE[key] = _build_program(*key)
    nc = _CACHE[key]

    if use_mask:
        # Reproduce the reference exactly: sort sequences by cluster id
        # (stable, as jnp.argsort), keep mask in unsorted order.
        cid2 = np.concatenate([np.asarray(cluster_id), np.asarray(cluster_id)])
        sidx = np.argsort(cid2, kind="stable")
        xs = seq[sidx]
    else:
        xs = seq  # sort o unsort == identity for batch-independent attention

    bf = ml_dtypes.bfloat16
    x_flat = xs.reshape(N_FULL * C, E)
    base = {
        "wqt": np.ascontiguousarray(Wq.T).astype(bf),
        "wkt": np.ascontiguousarray(Wk.T).astype(bf),
        "wvt": np.ascontiguousarray(Wv.T).astype(bf),
        "wdt": np.ascontiguousarray(Wd.T).astype(bf),
    }
    if use_bq:
        base["bq"] = bq
    if use_bk:
        base["bk"] = bk
    if use_bv:
        base["bv"] = bv
    if use_bd:
        base["bdb"] = np.ascontiguousarray(np.tile(bd[None, :], (128, 1)))
    in_maps = []
    for i in range(N_CORES):
        im = dict(base)
        xi = np.ascontiguousarray(x_flat[i * T_FULL : (i + 1) * T_FULL])
        im["x"] = xi
        im["xt"] = np.ascontiguousarray(xi.T).astype(bf)
        if use_mask:
            im["mask"] = np.ascontiguousarray(
                attention_mask[i * NSH : (i + 1) * NSH, 0, :, :].reshape(T_FULL, C)
            )
        in_maps.append(im)

    import os

    trace = bool(int(os.environ.get("KERNEL_TRACE", "0")))
    res = bass_utils.run_bass_kernel_spmd(
        nc, in_maps, core_ids=list(range(N_CORES)), trace=trace
    )
    kernel._last_result = res

    out = np.concatenate([r["out"] for r in res.results], axis=0)
    out = out.reshape(N_FULL, C, E)
    if use_mask:
        out = out[np.argsort(sidx, kind="stable")]
    if not (np.all(ln_w == 1.0) and np.all(ln_b == 0.0)):
        out = out * ln_w + ln_b
    return out.astype(np.float32)
